# revision 2
# baseline (speedup 1.0000x reference)
"""Causal cosine-sim attention (qk rmsnorm, scale=8) on 8 trn2 NeuronCores.

Shapes: q,k,v [2,16,2048,64] fp32; out [2,16,2048,64] fp32.
Sharding: 32 (batch, head) pairs -> 4 per core (head-parallel, per the
sharding hint); each core runs an identical program on its own 4 heads.

Per-core algorithm (per head):
  preprocess: load Q/K [s,d], l2-normalize rows (DVE sq+reduce, ACT
    ln/exp(-0.5x) = rsqrt), cast to bf16, round-trip through a DRAM
    scratch [s,128] and DMA-xbar-transpose back as qT/kT [d, s] bf16,
    then apply q_scale/k_scale (per-partition now).  V loads as [s,d],
    cast bf16 with a ones-column appended (rowsum trick).
  attention (j-major): for each key block jb (128 keys), S^T tail
    tile = kT_jb.T @ qT over the causal i-range in <=1024-col chunks
    (PSUM, bf16 matmul), ACT exp(8*x + mask_bias) PSUM->SBUF bf16,
    multiply the diagonal 128x128 block by a lower-tri mask, then
    accumulate O^T[65, i] += V_jb.T @ P^T in PSUM (col 64 = rowsum).
  epilogue: copy O^T to SBUF, PE-transpose per 128-col tile, divide by
    rowsum (DVE reciprocal + tensor_scalar), DMA out.
"""

import sys

import numpy as np

try:
    import concourse.bass as bass
except ImportError:
    sys.path.insert(0, "/opt/trn_rl_repo")
    import concourse.bass as bass

import concourse.mybir as mybir
import concourse.tile as tile
from concourse import bacc
from concourse.bass_utils import run_bass_kernel_spmd
from concourse.masks import make_identity

FP32 = mybir.dt.float32
BF16 = mybir.dt.bfloat16

N_CORES = 8
B, H, S, D = 2, 16, 2048, 64
HPC = (B * H) // N_CORES  # heads per core = 4
P = 128
NT = S // P  # 16 key/query blocks
COSINE_SIM_SCALE = 8.0
MASK_NEG = -1e30


def build_nc():
    nc = bacc.Bacc("TRN2", target_bir_lowering=False, debug=False)

    q_d = nc.dram_tensor("q", [HPC, S, D], FP32, kind="ExternalInput")
    k_d = nc.dram_tensor("k", [HPC, S, D], FP32, kind="ExternalInput")
    v_d = nc.dram_tensor("v", [HPC, S, D], FP32, kind="ExternalInput")
    qs_d = nc.dram_tensor("q_scale", [D], FP32, kind="ExternalInput")
    ks_d = nc.dram_tensor("k_scale", [D], FP32, kind="ExternalInput")
    mb_d = nc.dram_tensor("mbias", [HPC, S], FP32, kind="ExternalInput")
    out_d = nc.dram_tensor("out", [HPC, S, D], FP32, kind="ExternalOutput")

    AF = mybir.ActivationFunctionType
    ALU = mybir.AluOpType

    with tile.TileContext(nc) as tc:
        with (
            tc.tile_pool(name="constp", bufs=1) as constp,
            tc.tile_pool(name="dramp", bufs=4, space="DRAM") as dramp,
            tc.tile_pool(name="stagep", bufs=3) as stagep,
            tc.tile_pool(name="sqp", bufs=2) as sqp,
            tc.tile_pool(name="ssp", bufs=6) as ssp,
            tc.tile_pool(name="qnp", bufs=3) as qnp,
            tc.tile_pool(name="qtp", bufs=2) as qtp,
            tc.tile_pool(name="ktp", bufs=2) as ktp,
            tc.tile_pool(name="vbp", bufs=2) as vbp,
            tc.tile_pool(name="mbp", bufs=2) as mbp,
            tc.tile_pool(name="ptp", bufs=3) as ptp,
            tc.tile_pool(name="otsbp", bufs=2) as otsbp,
            tc.tile_pool(name="osbp", bufs=2) as osbp,
            tc.tile_pool(name="recp", bufs=4) as recp,
            tc.tile_pool(name="stp", bufs=2, space="PSUM") as stp,
            tc.tile_pool(name="otp", bufs=1, space="PSUM") as otp,
        ):
            # ---- constants ----
            tri = constp.tile([P, P], BF16, name="tri")
            nc.gpsimd.memset(tri[:], 1.0)
            # keep where col >= row (P^T layout: row=key j, col=query i)
            nc.gpsimd.affine_select(
                out=tri[:],
                in_=tri[:],
                pattern=[[1, P]],
                channel_multiplier=-1,
                base=0,
                compare_op=ALU.is_ge,
                fill=0.0,
            )
            ident = constp.tile([P, P], FP32, name="ident")
            make_identity(nc, ident[:])
            qscale_sb = constp.tile([D, 1], FP32, name="qscale_sb")
            nc.sync.dma_start(out=qscale_sb[:], in_=qs_d[:].rearrange("(d one) -> d one", one=1))
            kscale_sb = constp.tile([D, 1], FP32, name="kscale_sb")
            nc.sync.dma_start(out=kscale_sb[:], in_=ks_d[:].rearrange("(d one) -> d one", one=1))

            def normalize_transpose(h, src_d, scale_sb, which):
                """-> qT/kT tile [128, S] bf16 (rows 0:D valid)."""
                xs = stagep.tile([P, NT * D], FP32, tag="stage", name=f"xs_{which}{h}")
                nc.sync.dma_start(
                    out=xs.rearrange("p (t d) -> p t d", d=D),
                    in_=src_d[h].rearrange("(t p) d -> p t d", p=P),
                )
                sq = sqp.tile([P, NT * D], FP32, tag="sq", name=f"sq_{which}{h}")
                nc.vector.tensor_mul(sq[:], xs[:], xs[:])
                ss = ssp.tile([P, NT], FP32, tag="ss", name=f"ss_{which}{h}")
                nc.vector.tensor_reduce(
                    out=ss[:],
                    in_=sq.rearrange("p (t d) -> p t d", d=D),
                    axis=mybir.AxisListType.X,
                    op=ALU.add,
                )
                lnss = ssp.tile([P, NT], FP32, tag="ss", name=f"ln_{which}{h}")
                nc.scalar.activation(lnss[:], ss[:], AF.Ln)
                rs = ssp.tile([P, NT], FP32, tag="ss", name=f"rs_{which}{h}")
                # rsqrt(ss) = exp(-0.5 * ln(ss)); Rsqrt ACT table is banned.
                nc.scalar.activation(rs[:], lnss[:], AF.Exp, scale=-0.5)
                xn = qnp.tile([P, NT * D], BF16, tag="qn", name=f"xn_{which}{h}")
                for t in range(NT):
                    nc.vector.tensor_scalar(
                        xn[:, t * D : (t + 1) * D],
                        xs[:, t * D : (t + 1) * D],
                        rs[:, t : t + 1],
                        None,
                        ALU.mult,
                    )
                scratch = dramp.tile([S, P], BF16, tag="scratch", name=f"sc_{which}{h}")
                nc.sync.dma_start(
                    out=scratch.rearrange("(t p) c -> p t c", p=P)[:, :, 0:D],
                    in_=xn.rearrange("p (t d) -> p t d", d=D),
                )
                pool = qtp if which == "q" else ktp
                xt = pool.tile([P, S], BF16, tag=f"{which}T", name=f"{which}T{h}")
                nc.sync.dma_start_transpose(out=xt[:], in_=scratch[:])
                nc.vector.tensor_scalar(
                    xt[0:D, :], xt[0:D, :], scale_sb[:, 0:1], None, ALU.mult
                )
                return xt

            for h in range(HPC):
                # ---- phase A: preprocess ----
                qT = normalize_transpose(h, q_d, qscale_sb, "q")
                kT = normalize_transpose(h, k_d, kscale_sb, "k")

                vs = stagep.tile([P, NT * D], FP32, tag="stage", name=f"vs{h}")
                nc.sync.dma_start(
                    out=vs.rearrange("p (t d) -> p t d", d=D),
                    in_=v_d[h].rearrange("(t p) d -> p t d", p=P),
                )
                vb = vbp.tile([P, NT * (D + 1)], BF16, tag="vb", name=f"vb{h}")
                for t in range(NT):
                    nc.gpsimd.tensor_copy(
                        vb[:, t * (D + 1) : t * (D + 1) + D],
                        vs[:, t * D : (t + 1) * D],
                    )
                nc.gpsimd.memset(
                    vb.rearrange("p (t c) -> p t c", c=D + 1)[:, :, D : D + 1], 1.0
                )
                mbias = mbp.tile([P, NT], FP32, tag="mb", name=f"mb{h}")
                nc.sync.dma_start(
                    out=mbias[:], in_=mb_d[h].rearrange("(t p) -> p t", p=P)
                )

                # ---- phase B: attention, j-major over key blocks ----
                oT = otp.tile([D + 1, S], FP32, tag="oT", name=f"oT{h}")
                for jb in range(NT):
                    i0 = jb * P
                    chunks = []
                    if i0 < S // 2:
                        chunks.append((i0, S // 2))
                    chunks.append((max(i0, S // 2), S))
                    first = True
                    for cs, ce in chunks:
                        W = ce - cs
                        st = stp.tile([P, W], FP32, tag="st", name=f"st{h}_{jb}_{cs}")
                        n0 = cs
                        while n0 < ce:
                            w = min(512, ce - n0)
                            nc.tensor.matmul(
                                st[:, n0 - cs : n0 - cs + w],
                                kT[0:D, i0 : i0 + P],
                                qT[0:D, n0 : n0 + w],
                                start=True,
                                stop=True,
                            )
                            n0 += w
                        pT = ptp.tile([P, W], BF16, tag="pT", name=f"pT{h}_{jb}_{cs}")
                        nc.scalar.activation(
                            pT[:],
                            st[:],
                            AF.Exp,
                            scale=COSINE_SIM_SCALE,
                            bias=mbias[:, jb : jb + 1],
                        )
                        if first:
                            # zero the strictly-lower (i < j) part of the
                            # diagonal 128x128 block
                            nc.gpsimd.tensor_mul(pT[:, 0:P], pT[:, 0:P], tri[:])
                            first = False
                        n0 = cs
                        while n0 < ce:
                            w = min((n0 // 512 + 1) * 512, ce) - n0
                            bank = n0 // 512
                            nc.tensor.matmul(
                                oT[:, n0 : n0 + w],
                                vb[:, jb * (D + 1) : (jb + 1) * (D + 1)],
                                pT[:, n0 - cs : n0 - cs + w],
                                start=(jb == 0),
                                stop=(jb == min(NT - 1, 4 * bank + 3)),
                                skip_group_check=True,
                            )
                            n0 += w

                # ---- phase C: transpose + divide + store ----
                oT_sb = otsbp.tile([D + 1, S], FP32, tag="otsb", name=f"osb_t{h}")
                nc.vector.tensor_copy(oT_sb[:], oT[:])
                osb = osbp.tile([P, NT * D], FP32, tag="osb", name=f"osb{h}")
                for ib in range(NT):
                    tp = stp.tile([P, D + 1], FP32, tag="st", name=f"tp{h}_{ib}")
                    nc.tensor.transpose(
                        tp[:],
                        oT_sb[:, ib * P : (ib + 1) * P],
                        ident[0 : D + 1, 0 : D + 1],
                    )
                    rec = recp.tile([P, 1], FP32, tag="rec", name=f"rec{h}_{ib}")
                    nc.vector.reciprocal(rec[:], tp[:, D : D + 1])
                    nc.vector.tensor_scalar(
                        osb[:, ib * D : (ib + 1) * D],
                        tp[:, 0:D],
                        rec[:, 0:1],
                        None,
                        ALU.mult,
                    )
                nc.sync.dma_start(
                    out=out_d[h].rearrange("(t p) d -> p t d", p=P),
                    in_=osb.rearrange("p (t d) -> p t d", d=D),
                )

    nc.compile()
    return nc


_NC_CACHE = None


def kernel(q, k, v, q_scale, k_scale, mask):
    global _NC_CACHE
    q = np.asarray(q, dtype=np.float32)
    k = np.asarray(k, dtype=np.float32)
    v = np.asarray(v, dtype=np.float32)
    q_scale = np.asarray(q_scale, dtype=np.float32)
    k_scale = np.asarray(k_scale, dtype=np.float32)
    mask = np.asarray(mask)

    qf = q.reshape(B * H, S, D)
    kf = k.reshape(B * H, S, D)
    vf = v.reshape(B * H, S, D)
    # additive key-padding bias per (b,h) row, matching reference's where()
    mbias_bh = np.where(mask, 0.0, MASK_NEG).astype(np.float32)  # [B, S]

    if _NC_CACHE is None:
        _NC_CACHE = build_nc()
    nc = _NC_CACHE

    in_maps = []
    for c in range(N_CORES):
        heads = list(range(c * HPC, (c + 1) * HPC))
        in_maps.append(
            {
                "q": np.ascontiguousarray(qf[heads]),
                "k": np.ascontiguousarray(kf[heads]),
                "v": np.ascontiguousarray(vf[heads]),
                "q_scale": q_scale,
                "k_scale": k_scale,
                "mbias": np.ascontiguousarray(
                    np.stack([mbias_bh[bh // H] for bh in heads])
                ),
            }
        )

    res = run_bass_kernel_spmd(nc, in_maps, core_ids=list(range(N_CORES)))
    out = np.stack([r["out"] for r in res.results])  # [8, 4, S, D]
    return out.reshape(B, H, S, D).astype(np.float32)


# revision 4
# speedup vs baseline: 1.1350x; 1.1350x over previous
"""Causal cosine-sim attention (qk rmsnorm, scale=8) on 8 trn2 NeuronCores.

Shapes: q,k,v [2,16,2048,64] fp32; out [2,16,2048,64] fp32.
Sharding: 32 (batch, head) pairs -> 4 per core (head-parallel, per the
sharding hint); each core runs an identical program on its own 4 heads.

Per-core algorithm (per head):
  preprocess: load Q/K [s,d], l2-normalize rows (DVE sq+reduce, ACT
    ln + exp(-0.5x) = rsqrt, batched across heads so the ACT table set
    loads only twice), cast to bf16, round-trip through a DRAM scratch
    [s,128] written twice (cols 0:64 and 64:128) and DMA-xbar-transpose
    back as qT/kT [128, s] bf16 whose two partition halves are copies —
    the duplicate feeds row-group-packed matmuls.  V loads as [s,d],
    cast bf16 with a ones-column appended (rowsum trick).
  attention (j-major, i-halves of 1024): for each key-block pair
    (jb, jb+1), S^T tiles = kT_jb.T @ qT over the causal i-range inside
    the half; the pair runs on disjoint PE row-groups (partitions 0:64
    and 64:128) so the two matmuls overlap.  ACT exp(8*x + mask_bias)
    PSUM->SBUF bf16; the diagonal 128x128 block is separately multiplied
    by a lower-tri mask into its own small tile (GpSimd) so the rest of
    the PV chain never waits on it; O^T[65, i-half] += V_jb.T @ P^T
    accumulates in PSUM (col 64 = rowsum).
  epilogue per half: copy O^T to SBUF (DVE), PE-transpose per 128-col
    tile, divide by rowsum (DVE reciprocal + tensor_scalar), DMA out.
"""

import sys

import numpy as np

try:
    import concourse.bass as bass
except ImportError:
    sys.path.insert(0, "/opt/trn_rl_repo")
    import concourse.bass as bass

import concourse.mybir as mybir
import concourse.tile as tile
from concourse import bacc
from concourse.bass_utils import run_bass_kernel_spmd
from concourse.masks import make_identity

FP32 = mybir.dt.float32
BF16 = mybir.dt.bfloat16

N_CORES = 8
B, H, S, D = 2, 16, 2048, 64
HPC = (B * H) // N_CORES  # heads per core = 4
P = 128
NT = S // P  # 16 key/query blocks
HALF = S // 2
COSINE_SIM_SCALE = 8.0
MASK_NEG = -1e30


def build_nc():
    nc = bacc.Bacc("TRN2", target_bir_lowering=False, debug=False)

    q_d = nc.dram_tensor("q", [HPC, S, D], FP32, kind="ExternalInput")
    k_d = nc.dram_tensor("k", [HPC, S, D], FP32, kind="ExternalInput")
    v_d = nc.dram_tensor("v", [HPC, S, D], FP32, kind="ExternalInput")
    qs_d = nc.dram_tensor("q_scale", [D], FP32, kind="ExternalInput")
    ks_d = nc.dram_tensor("k_scale", [D], FP32, kind="ExternalInput")
    mb_d = nc.dram_tensor("mbias", [HPC, S], FP32, kind="ExternalInput")
    out_d = nc.dram_tensor("out", [HPC, S, D], FP32, kind="ExternalOutput")

    AF = mybir.ActivationFunctionType
    ALU = mybir.AluOpType

    with tile.TileContext(nc) as tc:
        with (
            tc.tile_pool(name="constp", bufs=1) as constp,
            tc.tile_pool(name="dramp", bufs=4, space="DRAM") as dramp,
            tc.tile_pool(name="stagep", bufs=7) as stagep,
            tc.tile_pool(name="sqp", bufs=2) as sqp,
            tc.tile_pool(name="ssp", bufs=12) as ssp,
            tc.tile_pool(name="qnp", bufs=4) as qnp,
            tc.tile_pool(name="qtp", bufs=3) as qtp,
            tc.tile_pool(name="ktp", bufs=3) as ktp,
            tc.tile_pool(name="vbp", bufs=3) as vbp,
            tc.tile_pool(name="mbp", bufs=3) as mbp,
            tc.tile_pool(name="ptp", bufs=4) as ptp,
            tc.tile_pool(name="dtp", bufs=4) as dtp,
            tc.tile_pool(name="otsbp", bufs=2) as otsbp,
            tc.tile_pool(name="osbp", bufs=2) as osbp,
            tc.tile_pool(name="recp", bufs=8) as recp,
            tc.tile_pool(name="stp", bufs=3, space="PSUM") as stp,
            tc.tile_pool(name="otp", bufs=1, space="PSUM") as otp,
        ):
            # ---- constants ----
            tri = constp.tile([P, P], BF16, name="tri")
            nc.gpsimd.memset(tri[:], 1.0)
            # keep where col >= row (P^T layout: row=key j, col=query i)
            nc.gpsimd.affine_select(
                out=tri[:],
                in_=tri[:],
                pattern=[[1, P]],
                channel_multiplier=-1,
                base=0,
                compare_op=ALU.is_ge,
                fill=0.0,
            )
            ident = constp.tile([P, P], FP32, name="ident")
            make_identity(nc, ident[:])
            # q/k per-dim scales duplicated over both partition halves so
            # one tensor_scalar covers the row-packed qT/kT copies
            qscale_sb = constp.tile([P, 1], FP32, name="qscale_sb")
            kscale_sb = constp.tile([P, 1], FP32, name="kscale_sb")
            for half in range(2):
                nc.sync.dma_start(
                    out=qscale_sb[half * D : (half + 1) * D, 0:1],
                    in_=qs_d[:].rearrange("(d one) -> d one", one=1),
                )
                nc.sync.dma_start(
                    out=kscale_sb[half * D : (half + 1) * D, 0:1],
                    in_=ks_d[:].rearrange("(d one) -> d one", one=1),
                )

            # ================= phase A (grouped by head pairs so the ACT
            # Ln/Exp table sets load at most twice per group) ============
            def load_stage(h, src_d, which):
                xs = stagep.tile([P, NT * D], FP32, tag="stage", name=f"xs_{which}{h}")
                nc.sync.dma_start(
                    out=xs.rearrange("p (t d) -> p t d", d=D),
                    in_=src_d[h].rearrange("(t p) d -> p t d", p=P),
                )
                return xs

            def norm_stats(h, xs, which):
                sq = sqp.tile([P, NT * D], FP32, tag="sq", name=f"sq_{which}{h}")
                nc.vector.tensor_mul(sq[:], xs[:], xs[:])
                ss = ssp.tile([P, NT], FP32, tag="ss", name=f"ss_{which}{h}")
                nc.vector.tensor_reduce(
                    out=ss[:],
                    in_=sq.rearrange("p (t d) -> p t d", d=D),
                    axis=mybir.AxisListType.X,
                    op=ALU.add,
                )
                return ss

            def finish_transpose(h, xs, rs, scale_sb, which, on_gpsimd):
                xn = qnp.tile([P, NT * D], BF16, tag="qn", name=f"xn_{which}{h}")
                rs_b = rs.rearrange("p (t one) -> p t one", one=1).broadcast_to(
                    [P, NT, D]
                )
                eng = nc.gpsimd if on_gpsimd else nc.vector
                eng.tensor_mul(
                    xn.rearrange("p (t d) -> p t d", d=D),
                    xs.rearrange("p (t d) -> p t d", d=D),
                    rs_b,
                )
                scratch = dramp.tile([S, P], BF16, tag="scratch", name=f"sc_{which}{h}")
                for half in range(2):
                    nc.sync.dma_start(
                        out=scratch.rearrange("(t p) c -> p t c", p=P)[
                            :, :, half * D : (half + 1) * D
                        ],
                        in_=xn.rearrange("p (t d) -> p t d", d=D),
                    )
                pool = qtp if which == "q" else ktp
                xt = pool.tile([P, S], BF16, tag=f"{which}T", name=f"{which}T{h}")
                nc.sync.dma_start_transpose(out=xt[:], in_=scratch[:])
                nc.vector.tensor_scalar(
                    xt[:], xt[:], scale_sb[:, 0:1], None, ALU.mult
                )
                return xt

            def prep_v(h):
                vs = stagep.tile([P, NT * D], FP32, tag="stage", name=f"vs{h}")
                nc.sync.dma_start(
                    out=vs.rearrange("p (t d) -> p t d", d=D),
                    in_=v_d[h].rearrange("(t p) d -> p t d", p=P),
                )
                vb = vbp.tile([P, NT * (D + 1)], BF16, tag="vb", name=f"vb{h}")
                nc.vector.tensor_copy(
                    vb.rearrange("p (t c) -> p t c", c=D + 1)[:, :, 0:D],
                    vs.rearrange("p (t d) -> p t d", d=D),
                )
                nc.gpsimd.memset(
                    vb.rearrange("p (t c) -> p t c", c=D + 1)[:, :, D : D + 1], 1.0
                )
                mbias = mbp.tile([P, NT], FP32, tag="mb", name=f"mb{h}")
                nc.sync.dma_start(
                    out=mbias[:], in_=mb_d[h].rearrange("(t p) -> p t", p=P)
                )
                return vb, mbias

            def preprocess_group(heads):
                """Batch the ACT Ln's then Exp's across the group."""
                staged = {}
                for h in heads:
                    staged[("q", h)] = load_stage(h, q_d, "q")
                    staged[("k", h)] = load_stage(h, k_d, "k")
                stats = {}
                for (which, h), xs in staged.items():
                    stats[(which, h)] = norm_stats(h, xs, which)
                lns = {}
                for key, ss in stats.items():
                    ln = ssp.tile([P, NT], FP32, tag="ss", name=f"ln_{key[0]}{key[1]}")
                    nc.scalar.activation(ln[:], ss[:], AF.Ln)
                    lns[key] = ln
                rss = {}
                for key, ln in lns.items():
                    rs = ssp.tile([P, NT], FP32, tag="ss", name=f"rs_{key[0]}{key[1]}")
                    nc.scalar.activation(rs[:], ln[:], AF.Exp, scale=-0.5)
                    rss[key] = rs
                outs = {}
                for (which, h), xs in staged.items():
                    outs[(which, h)] = finish_transpose(
                        h,
                        xs,
                        rss[(which, h)],
                        qscale_sb if which == "q" else kscale_sb,
                        which,
                        on_gpsimd=(which == "k"),
                    )
                vbs = {h: prep_v(h) for h in heads}
                return {
                    h: (outs[("q", h)], outs[("k", h)], *vbs[h]) for h in heads
                }

            def attention_head(h, qT, kT, vb, mbias):
                for ih in range(2):
                    ilo = ih * HALF
                    njb = (ilo + HALF) // P  # 8 or 16
                    oTh = otp.tile([D + 1, HALF], FP32, tag="oT", name=f"oT{h}_{ih}")
                    pts = []  # deferred PV work per jb in the pair
                    for jb in range(njb):
                        half_sel = jb % 2  # row-group for S^T packing
                        lo = half_sel * D
                        cs = max(jb * P, ilo)
                        ce = ilo + HALF
                        W = ce - cs
                        st = stp.tile(
                            [P, W], FP32, tag="st", name=f"st{h}_{ih}_{jb}"
                        )
                        n0 = cs
                        while n0 < ce:
                            w = min(512, ce - n0)
                            nc.tensor.matmul(
                                st[:, n0 - cs : n0 - cs + w],
                                kT[lo : lo + D, jb * P : (jb + 1) * P],
                                qT[lo : lo + D, n0 : n0 + w],
                                start=True,
                                stop=True,
                            )
                            n0 += w
                        pT = ptp.tile([P, W], BF16, tag="pT", name=f"pT{h}_{ih}_{jb}")
                        nc.scalar.activation(
                            pT[:],
                            st[:],
                            AF.Exp,
                            scale=COSINE_SIM_SCALE,
                            bias=mbias[:, jb : jb + 1],
                        )
                        has_diag = cs == jb * P
                        pTd = None
                        if has_diag:
                            pTd = dtp.tile([P, P], BF16, tag="pTd", name=f"pTd{h}_{ih}_{jb}")
                            nc.gpsimd.tensor_mul(pTd[:], pT[:, 0:P], tri[:])
                        # ---- PV accumulation ----
                        vslice = vb[:, jb * (D + 1) : (jb + 1) * (D + 1)]
                        n0 = cs
                        while n0 < ce:
                            rel = n0 - ilo
                            w = min(ilo + (rel // 512 + 1) * 512, ce) - n0
                            bank = rel // 512
                            last_jb = (ilo + 512 * bank + 511) // P
                            if has_diag and n0 == cs:
                                w = P  # diagonal block from masked tile
                                rhs = pTd[:]
                            else:
                                rhs = pT[:, n0 - cs : n0 - cs + w]
                            nc.tensor.matmul(
                                oTh[:, rel : rel + w],
                                vslice,
                                rhs,
                                start=(jb == 0),
                                stop=(jb == last_jb),
                                skip_group_check=True,
                            )
                            n0 += w

                    # ---- epilogue for this half ----
                    oT_sb = otsbp.tile(
                        [D + 1, HALF], FP32, tag="otsb", name=f"otsb{h}_{ih}"
                    )
                    nc.vector.tensor_copy(oT_sb[:], oTh[:])
                    osb = osbp.tile([P, HALF // 2], FP32, tag="osb", name=f"osb{h}_{ih}")
                    nt_h = HALF // P  # 8 tiles per half
                    for ib in range(nt_h):
                        tp = stp.tile([P, D + 1], FP32, tag="st", name=f"tp{h}_{ih}_{ib}")
                        nc.tensor.transpose(
                            tp[:],
                            oT_sb[:, ib * P : (ib + 1) * P],
                            ident[0 : D + 1, 0 : D + 1],
                        )
                        rec = recp.tile([P, 1], FP32, tag="rec", name=f"rec{h}_{ih}_{ib}")
                        nc.vector.reciprocal(rec[:], tp[:, D : D + 1])
                        nc.vector.tensor_scalar(
                            osb[:, ib * D : (ib + 1) * D],
                            tp[:, 0:D],
                            rec[:, 0:1],
                            None,
                            ALU.mult,
                        )
                    nc.sync.dma_start(
                        out=out_d[h].rearrange("(t p) d -> p t d", p=P)[
                            :, ih * nt_h : (ih + 1) * nt_h, :
                        ],
                        in_=osb.rearrange("p (t d) -> p t d", d=D),
                    )

            # group heads in pairs: (h0,h1) preprocess, attend h0, h1 while
            # (h2,h3) preprocess, etc.  4 ACT table loads total.
            g0 = preprocess_group([0, 1])
            attention_head(0, *g0[0])
            g1 = preprocess_group([2, 3])
            attention_head(1, *g0[1])
            attention_head(2, *g1[2])
            attention_head(3, *g1[3])

    nc.compile()
    return nc


_NC_CACHE = None


def kernel(q, k, v, q_scale, k_scale, mask):
    global _NC_CACHE
    q = np.asarray(q, dtype=np.float32)
    k = np.asarray(k, dtype=np.float32)
    v = np.asarray(v, dtype=np.float32)
    q_scale = np.asarray(q_scale, dtype=np.float32)
    k_scale = np.asarray(k_scale, dtype=np.float32)
    mask = np.asarray(mask)

    qf = q.reshape(B * H, S, D)
    kf = k.reshape(B * H, S, D)
    vf = v.reshape(B * H, S, D)
    # additive key-padding bias per (b,h) row, matching reference's where()
    mbias_bh = np.where(mask, 0.0, MASK_NEG).astype(np.float32)  # [B, S]

    if _NC_CACHE is None:
        _NC_CACHE = build_nc()
    nc = _NC_CACHE

    in_maps = []
    for c in range(N_CORES):
        heads = list(range(c * HPC, (c + 1) * HPC))
        in_maps.append(
            {
                "q": np.ascontiguousarray(qf[heads]),
                "k": np.ascontiguousarray(kf[heads]),
                "v": np.ascontiguousarray(vf[heads]),
                "q_scale": q_scale,
                "k_scale": k_scale,
                "mbias": np.ascontiguousarray(
                    np.stack([mbias_bh[bh // H] for bh in heads])
                ),
            }
        )

    res = run_bass_kernel_spmd(nc, in_maps, core_ids=list(range(N_CORES)))
    out = np.stack([r["out"] for r in res.results])  # [8, 4, S, D]
    return out.reshape(B, H, S, D).astype(np.float32)
